# revision 3
# baseline (speedup 1.0000x reference)
"""Criss-cross attention (nn_CC_attention) Trainium2 kernel, v2.

Sharding: pure data parallel over batch B=8 across 8 NeuronCores; the only
cross-core coupling is the global min/max of energy (AllGather of (max,-min)
+ local max-reduce).

Host staging (layout/precision only):
  t1b = fp8(tensor1)  as (H, C, W)
  t2h = fp8(tensor2)  as (H, C, W)
  t2t = fp16(tensor2) as (W, C, H)   -- carries the exact residual
  out is produced as fp16 (W, C, H); host transposes back to (C, H, W) fp32.

Per-core algorithm (all moving operands are the BIG tensors so the PE streams
at full rate; stationaries are either tiny (16-col keys) or fp8 (fast FWL)):

  phase 1, per channel c (streamed in 32-channel chunks, 16-channel groups):
    kH[c][w,k]  = t1b[c].T @ pool16            (PE, N=16)
    GW[w',w]   += t1b[c].T @ t2h[c]            (PE, N=128, fp8; gram matrix)
    eHT[k,h]   += kh16[c].T @ t2t[c]           (PE, N=128, 16-col stationary)
  boundary:
    eWT[k,w] = pool16b.T @ bf16(GW)            (one matmul; pools the gram)
    local (max,-min) of e=[eHT|eWT] -> AllGather(8x2 scalars) -> global range
    att = softmax((e - gmin)/R)  (exp on ACT with accum; sums via gpsimd)
    [MHT | MWc] = expmat.T @ att2  (one matmul, N=256)
      MHT[h',h] = 0.0625*att_H[h, h'//8]   (bf16)
      MWI[w',w] = 0.0625*att_W[w, w'//8] + I  (bf16; residual rides the I)
  phase 2, per channel c (batches of 8, psum [W, 8H]):
    ps[w,h]  = t2h[c].T @ MHT                  (= 0.5*out_H^T)
    ps[w,h] += MWI.T @ t2t[c]                  (+ 0.5*out_W^T + t2[c]^T)
    copy ps -> fp16 ob (split ACT/DVE/GpSimd), chunked DMA out.

PE warm-up bursts keep the HAM clock gate open; a dependency-free warm-up
AllGather at kernel start absorbs the first-collective barrier cost.
"""

import numpy as np
from contextlib import ExitStack

import ml_dtypes
import concourse.bass as bass
import concourse.tile as tile
from concourse import bacc, bass_isa, mybir

B, C, H, W, POOL = 8, 256, 128, 128, 8
KH, KW = H // POOL, W // POOL  # 16, 16
NCORES = 8
G = 16       # channels per compute group
CHUNK = 32   # channels per DMA chunk (2 groups)
NWARM1 = 50  # dummy matmuls in the kernel-start PE warm-up burst
NWARM2 = 36  # dummy matmuls in the post-collective PE warm-up burst

F32 = mybir.dt.float32
F16 = mybir.dt.float16
BF16 = mybir.dt.bfloat16
F8 = mybir.dt.float8e4
BF_NP = ml_dtypes.bfloat16
F8_NP = ml_dtypes.float8_e4m3


def host_constants():
    pool_m = np.zeros((H, KH), np.float32)
    for k in range(KH):
        pool_m[k * POOL:(k + 1) * POOL, k] = 1.0 / POOL
    expmat = np.zeros((KH, H), np.float32)
    for k in range(KH):
        expmat[k, k * POOL:(k + 1) * POOL] = 0.5 / POOL  # 0.0625 (gamma folded)
    return {
        "pool16": pool_m.astype(F8_NP),
        "pool16b": pool_m.astype(BF_NP),
        "expmat": expmat.astype(BF_NP),
        "eyefull": np.eye(H, dtype=np.float32),
    }


def build(c_total=C, ncores=NCORES):
    assert c_total % CHUNK == 0
    nchunks = c_total // CHUNK
    nc = bacc.Bacc(trn_type="TRN2", target_bir_lowering=False, debug=False,
                   num_devices=ncores)

    t1b = nc.dram_tensor("t1b", [H, c_total, W], F8, kind="ExternalInput").ap()
    t2h = nc.dram_tensor("t2h", [H, c_total, W], F8, kind="ExternalInput").ap()
    t2t = nc.dram_tensor("t2t", [W, c_total, H], F16, kind="ExternalInput").ap()
    pool16 = nc.dram_tensor("pool16", [H, KH], F8, kind="ExternalInput").ap()
    pool16b = nc.dram_tensor("pool16b", [H, KH], BF16, kind="ExternalInput").ap()
    expmat = nc.dram_tensor("expmat", [KH, H], BF16, kind="ExternalInput").ap()
    eyefull = nc.dram_tensor("eyefull", [H, W], F32, kind="ExternalInput").ap()
    out = nc.dram_tensor("out", [W, c_total, H], F16, kind="ExternalOutput").ap()

    with tile.TileContext(nc) as tc, ExitStack() as top:
        # ---- constants ----
        cpool = top.enter_context(tc.tile_pool(name="consts", bufs=1))
        c_pool16 = cpool.tile([H, KH], F8, tag="pool16")
        nc.sync.dma_start(c_pool16[:], pool16[:])
        c_pool16b = cpool.tile([H, KH], BF16, tag="pool16b")
        nc.sync.dma_start(c_pool16b[:], pool16b[:])
        c_expmat = cpool.tile([KH, H], BF16, tag="expmat")
        nc.sync.dma_start(c_expmat[:], expmat[:])
        c_eye = cpool.tile([H, W], F32, tag="eyefull")
        nc.sync.dma_start(c_eye[:], eyefull[:])

        # zero tile for PE warm-up (no data deps -> earliest possible issue)
        wtile0 = cpool.tile([H, W], BF16, tag="wtile0")
        nc.vector.memset(wtile0[:], 0.0)

        # t2 stays resident in SBUF for phase 2
        rpool = top.enter_context(tc.tile_pool(name="resident", bufs=1))
        t2g8 = rpool.tile([H, c_total * W], F8, tag="t2g8")
        t2tg = rpool.tile([W, c_total * H], F16, tag="t2tg")

        psb = ExitStack()  # psum pools released before phase 2
        ps_e = psb.enter_context(tc.tile_pool(name="ps_e", bufs=1, space="PSUM"))
        ps_et = ps_e.tile([KH, 2 * H], F32, tag="e")       # [:,0:H]=eHT, [:,H:]=eWT
        ps_gwp = psb.enter_context(tc.tile_pool(name="ps_gw", bufs=1, space="PSUM"))
        ps_gw = ps_gwp.tile([W, W], F32, tag="gw")
        ps_warm = psb.enter_context(tc.tile_pool(name="ps_warm", bufs=1, space="PSUM"))
        ps_w = ps_warm.tile([H, W], F32, tag="warm")

        spool = top.enter_context(tc.tile_pool(name="soft", bufs=1))
        dram = top.enter_context(tc.tile_pool(name="dram", bufs=1, space="DRAM"))

        # PE warm-up burst #1 (memset input only -> scheduled at kernel start)
        for _ in range(NWARM1):
            nc.tensor.matmul(ps_w[:], wtile0[:], wtile0[:], start=True, stop=True)

        # collective warm-up: dummy AllGather with an unwritten input tile, so
        # the doorbell fires immediately and the first-collective barrier cost
        # overlaps phase 1.
        wc_in = dram.tile([1, 8], F32, tag="wc_in")
        wc_out = dram.tile([ncores, 8], F32, tag="wc_out")
        nc.gpsimd.collective_compute(
            "AllGather", mybir.AluOpType.bypass,
            replica_groups=[list(range(ncores))],
            ins=[wc_in.opt()], outs=[wc_out.opt()],
        )

        # ================= phase 1 =================
        with ExitStack() as ph1:
            pin = ph1.enter_context(tc.tile_pool(name="pin", bufs=3))
            kpool = ph1.enter_context(tc.tile_pool(name="keys", bufs=3))
            ps_khp = ph1.enter_context(tc.tile_pool(name="ps_kh", bufs=2, space="PSUM"))

            for ch in range(nchunks):
                c0 = ch * CHUNK
                t1g = pin.tile([H, CHUNK * W], F8, tag="t1g")
                nc.sync.dma_start(t1g[:].rearrange("p (c w) -> p c w", c=CHUNK),
                                  t1b[:, c0:c0 + CHUNK, :])
                nc.sync.dma_start(
                    t2g8[:, c0 * W:(c0 + CHUNK) * W].rearrange(
                        "p (c w) -> p c w", c=CHUNK),
                    t2h[:, c0:c0 + CHUNK, :])
                nc.sync.dma_start(
                    t2tg[:, c0 * H:(c0 + CHUNK) * H].rearrange(
                        "p (c h) -> p c h", c=CHUNK),
                    t2t[:, c0:c0 + CHUNK, :])

                for s in range(CHUNK // G):
                    cg = c0 + s * G            # first channel of the group
                    qoff = s * G * W           # offset into t1g

                    # kH[c][w,k] = t1b[c].T @ pool16  (N=16)
                    ps_kh = ps_khp.tile([W, G * KH], F32, tag="kh")
                    for i in range(G):
                        nc.tensor.matmul(ps_kh[:, i * KH:(i + 1) * KH],
                                         t1g[:, qoff + i * W:qoff + (i + 1) * W],
                                         c_pool16[:], start=True, stop=True)

                    # GW[w',w] += t1b[c].T @ t2h[c]  (N=128, fp8)
                    for i in range(G):
                        c = cg + i
                        nc.tensor.matmul(ps_gw[:],
                                         t1g[:, qoff + i * W:qoff + (i + 1) * W],
                                         t2g8[:, c * W:(c + 1) * W],
                                         start=(c == 0), stop=(c == c_total - 1))

                    kh16 = kpool.tile([W, G * KH], F16, tag="kh16")
                    nc.vector.tensor_copy(kh16[:], ps_kh[:])

                    # eHT[k,h] += kh16[c].T @ t2t[c]  (N=128, 16-col stationary)
                    for i in range(G):
                        c = cg + i
                        nc.tensor.matmul(ps_et[:, 0:H],
                                         kh16[:, i * KH:(i + 1) * KH],
                                         t2tg[:, c * H:(c + 1) * H],
                                         start=(c == 0), stop=(c == c_total - 1))

        # ================= boundary =================
        # eWT[k,w] = pool16b.T @ bf16(GW)
        gw_sb = spool.tile([W, W], BF16, tag="gw_sb")
        nc.vector.tensor_copy(gw_sb[:], ps_gw[:])
        nc.tensor.matmul(ps_et[:, H:2 * H], c_pool16b[:], gw_sb[:],
                         start=True, stop=True)

        e2 = spool.tile([KH, 2 * H], F32, tag="e2")
        nc.vector.tensor_copy(e2[:], ps_et[:])

        # local (max, -min) packed as [16, 2]
        pack = spool.tile([KH, 2], F32, tag="pack")
        nc.vector.tensor_reduce(pack[:, 0:1], e2[:], axis=mybir.AxisListType.X,
                                op=mybir.AluOpType.max)
        rmin = spool.tile([KH, 1], F32, tag="rmin")
        nc.vector.tensor_reduce(rmin[:], e2[:], axis=mybir.AxisListType.X,
                                op=mybir.AluOpType.min)
        nc.vector.tensor_scalar_mul(pack[:, 1:2], rmin[:], -1.0)

        cc_in = dram.tile([1, 2 * KH], F32, tag="cc_in")
        cc_out = dram.tile([ncores, 2 * KH], F32, tag="cc_out")
        nc.scalar.dma_start(cc_in[:].rearrange("a (p x) -> (a p) x", p=KH),
                            pack[:])
        nc.gpsimd.collective_compute(
            "AllGather", mybir.AluOpType.bypass,
            replica_groups=[list(range(ncores))],
            ins=[cc_in.opt()], outs=[cc_out.opt()],
        )
        # all 8 cores' [16,2] packs -> [128, 2]; max over partitions
        g8 = spool.tile([ncores * KH, 2], F32, tag="g8")
        nc.scalar.dma_start(g8[:], cc_out[:].rearrange("c (p x) -> (c p) x", p=KH))
        g8r = spool.tile([ncores * KH, 2], F32, tag="g8r")
        nc.gpsimd.partition_all_reduce(g8r[:], g8[:], channels=ncores * KH,
                                       reduce_op=bass_isa.ReduceOp.max)

        # PE warm-up burst #2: gated on the AllGather result landing (g8) via
        # a DVE copy feeding a K=1 matmul; the rest WAW-chain on ps_w.
        g2b = spool.tile([1, 1], BF16, tag="g2b")
        nc.vector.tensor_copy(g2b[:], g8[0:1, 0:1])
        nc.tensor.matmul(ps_w[0:1, :], g2b[:], wtile0[0:1, :],
                         start=True, stop=True)
        for _ in range(NWARM2 - 1):
            nc.tensor.matmul(ps_w[:], wtile0[:], wtile0[:], start=True, stop=True)

        # softmax prep: scale = 1/R, bias = -gmin/R  (per-partition scalars)
        rng_t = spool.tile([ncores * KH, 1], F32, tag="rng")
        nc.vector.tensor_tensor(rng_t[:], g8r[:, 0:1], g8r[:, 1:2],
                                mybir.AluOpType.add)
        inv_t = spool.tile([ncores * KH, 1], F32, tag="inv")
        nc.vector.reciprocal(inv_t[:], rng_t[:])
        bias_t = spool.tile([ncores * KH, 1], F32, tag="bias")
        nc.vector.tensor_tensor(bias_t[:], g8r[:, 1:2], inv_t[:],
                                mybir.AluOpType.mult)

        s2 = spool.tile([KH, 2 * H], F32, tag="s2")
        ssum = spool.tile([KH, 1], F32, tag="ssum")
        nc.scalar.activation(s2[:], e2[:], mybir.ActivationFunctionType.Exp,
                             bias=bias_t[0:KH, :], scale=inv_t[0:KH, :],
                             accum_out=ssum[:])
        stot = spool.tile([KH, 1], F32, tag="stot")
        nc.gpsimd.partition_all_reduce(stot[:], ssum[:], channels=KH,
                                       reduce_op=bass_isa.ReduceOp.add)
        rn = spool.tile([KH, 1], F32, tag="rn")
        nc.vector.reciprocal(rn[:], stot[:])
        att2 = spool.tile([KH, 2 * H], BF16, tag="att2")
        nc.vector.tensor_scalar_mul(att2[:], s2[:], rn[:])

        # [MHT | MWc] = expmat.T @ att2  (N=256)
        apool = top.enter_context(tc.tile_pool(name="amats", bufs=1))
        with tc.tile_pool(name="ps_m", bufs=1, space="PSUM") as ps_mp:
            ps_m = ps_mp.tile([H, 2 * H], F32, tag="m")
            nc.tensor.matmul(ps_m[:], c_expmat[:], att2[:], start=True, stop=True)
            MHT = apool.tile([H, H], BF16, tag="MHT")
            nc.vector.tensor_copy(MHT[:], ps_m[:, 0:H])
            MWI = apool.tile([W, W], BF16, tag="MWI")
            nc.vector.scalar_tensor_tensor(MWI[:], ps_m[:, H:2 * H], 1.0,
                                           c_eye[:],
                                           op0=mybir.AluOpType.mult,
                                           op1=mybir.AluOpType.add)

        psb.close()

        # ================= phase 2 =================
        BAT = 8
        with ExitStack() as ph2:
            ps_out = ph2.enter_context(tc.tile_pool(name="ps_out", bufs=3, space="PSUM"))
            opool = ph2.enter_context(tc.tile_pool(name="outp", bufs=3))
            for ch in range(nchunks):
                c0 = ch * CHUNK
                ob = opool.tile([W, CHUNK * H], F16, tag="ob")
                for bs in range(CHUNK // BAT):
                    cb = c0 + bs * BAT
                    ps_o = ps_out.tile([W, BAT * H], F32, tag="ps_o")
                    for i in range(BAT):
                        c = cb + i
                        # 0.5*out_H^T : t2h[c].T @ MHT
                        nc.tensor.matmul(ps_o[:, i * H:(i + 1) * H],
                                         t2g8[:, c * W:(c + 1) * W],
                                         MHT[:], start=True, stop=False)
                    for i in range(BAT):
                        c = cb + i
                        # 0.5*out_W^T + t2^T : MWI.T @ t2t[c]
                        nc.tensor.matmul(ps_o[:, i * H:(i + 1) * H],
                                         MWI[:],
                                         t2tg[:, c * H:(c + 1) * H],
                                         start=False, stop=True)
                    off = bs * BAT * H
                    acut = 448  # ACT/DVE split by measured ns/col (1.42 vs 1.17)
                    nc.scalar.copy(ob[:, off:off + acut], ps_o[:, 0:acut])
                    nc.vector.tensor_copy(ob[:, off + acut:off + BAT * H],
                                          ps_o[:, acut:])
                nc.sync.dma_start(out[:, c0:c0 + CHUNK, :],
                                  ob[:].rearrange("p (c h) -> p c h", c=CHUNK))

    nc.compile()
    return nc


_NC_CACHE = {}


def _get_nc():
    key = (C, NCORES)
    if key not in _NC_CACHE:
        _NC_CACHE[key] = build(C, NCORES)
    return _NC_CACHE[key]


def _stage(tensor1, tensor2):
    """Host-side precision/layout staging for all cores."""
    t1b = np.ascontiguousarray(
        tensor1.astype(F8_NP).transpose(0, 2, 1, 3))            # (B,H,C,W) fp8
    t2h = np.ascontiguousarray(
        tensor2.astype(F8_NP).transpose(0, 2, 1, 3))            # (B,H,C,W) fp8
    t2t = np.ascontiguousarray(
        tensor2.astype(np.float16).transpose(0, 3, 1, 2))       # (B,W,C,H) fp16
    return t1b, t2h, t2t


def kernel(tensor1: np.ndarray, tensor2: np.ndarray) -> np.ndarray:
    from concourse.bass_utils import run_bass_kernel_spmd
    assert tensor1.shape == (B, C, H, W) and tensor2.shape == (B, C, H, W)
    nc = _get_nc()
    consts = host_constants()
    t1b, t2h, t2t = _stage(np.asarray(tensor1, np.float32),
                           np.asarray(tensor2, np.float32))
    in_maps = [
        {"t1b": t1b[b], "t2h": t2h[b], "t2t": t2t[b], **consts}
        for b in range(B)
    ]
    res = run_bass_kernel_spmd(nc, in_maps, core_ids=list(range(NCORES)))
    out_wch = np.stack([res.results[b]["out"] for b in range(B)])  # (B,W,C,H) f16
    return np.ascontiguousarray(
        out_wch.transpose(0, 2, 3, 1).astype(np.float32))
